# revision 2
# baseline (speedup 1.0000x reference)
"""4D Conv-MLP (conv3^4 -> ReLU -> conv3^4) on 8 Trainium2 NeuronCores.

Sharding: core = b*4 + j  (batch b in {0,1}, H-slab j in {0..3}, 8 output rows
each). Each core computes its output slab independently: conv1 is recomputed on
a 1-row h halo (10 h rows from 12 x rows), so no cross-core communication is
needed. One SPMD program for all cores; per-core boundary behavior is driven by
data (host-zeroed x halos + h halo-row masks).

On-chip algorithm (implicit GEMM over the 81 taps, fp16 operands, fp32 PSUM):
  - x is stored channel-on-partition as a zero-padded flat plane per t:
    [18 D][12 H][34 W] (+1 lead pad), duplicated on partitions 64..127 shifted
    by one element so each K=128 matmul contracts two W-taps at once.
  - conv1: per (t, d): 340-column matmuls accumulating (kt,ku,kv) pair+single
    taps; ReLU+bias on the Scalar engine writes fp16 h (pads skipped).
  - conv2: per (t, d-pair): 512-column matmuls over all valid taps with
    K=128; bias added on DVE; fp32 result DMAd out.
"""

import numpy as np

B, C_IN, C_HID, C_OUT = 2, 64, 128, 64
T, D, H, W = 4, 16, 32, 32
NCORES, NJ = 8, 4
SH = H // NJ          # 8 out rows per slab
XH = SH + 4           # 12 x rows per slab
HHH = SH + 2          # 10 h rows per slab
XROW = 34             # padded W
XDP = 12 * XROW       # 408
XP = 1 + 18 * XDP + 7   # x plane size = 7352
HD, HW_ = 18, 34
HP = HD * HHH * HW_   # h plane = 6120
N1 = HHH * XROW       # conv1 run = 340
N2 = 512              # conv2 run (2 d-rows)

_cache = {}


def _t_taps(t):
    return [kt for kt in range(3) if 0 <= t + kt - 1 < T]


def _g27(kt, ku, kv):
    return (kt * 3 + ku) * 3 + kv


def _g81(kt, ku, kv, kw):
    return ((kt * 3 + ku) * 3 + kv) * 3 + kw


def _make_host_arrays(x, w1, b1, w2, b2):
    x = np.asarray(x, np.float32)
    Xs, MTs, MBs = [], [], []
    for core in range(NCORES):
        b, j = divmod(core, NJ)
        h0 = SH * j
        slab = np.zeros((C_IN, T, D, XH, W), np.float32)
        lo, hi = h0 - 2, h0 + 10
        slo, shi = max(lo, 0), min(hi, H)
        slab[:, :, :, slo - lo:shi - lo, :] = x[b, :, :, :, slo:shi, :]
        plane = np.zeros((C_IN, T, HD, XH, XROW), np.float32)
        plane[:, :, 1:17, :, 1:33] = slab
        flat = plane.reshape(C_IN, T, 18 * XDP)
        X = np.zeros((128, T, XP), np.float16)
        X[:64, :, 1:1 + 18 * XDP] = flat
        X[64:, :, 0:XP - 1] = X[:64, :, 1:XP]
        Xs.append(X)
        MTs.append(np.full((128, 1), 0.0 if j == 0 else 1.0, np.float32))
        MBs.append(np.full((128, 1), 0.0 if j == NJ - 1 else 1.0, np.float32))

    w1 = np.asarray(w1, np.float32)
    w2 = np.asarray(w2, np.float32)
    W1P = np.zeros((128, 27, 128), np.float16)
    W1S = np.zeros((128, 27, 128), np.float16)
    for kt in range(3):
        for ku in range(3):
            for kv in range(3):
                g = _g27(kt, ku, kv)
                W1P[:64, g, :] = w1[:, :, kt, ku, kv, 0].T
                W1P[64:, g, :] = w1[:, :, kt, ku, kv, 1].T
                W1S[:64, g, :] = w1[:, :, kt, ku, kv, 2].T
                W1S[64:, g, :] = w1[:, :, kt, ku, kv, 2].T
    W2 = np.zeros((128, 81, 64), np.float16)
    for kt in range(3):
        for ku in range(3):
            for kv in range(3):
                for kw in range(3):
                    gi = _g81(kt, ku, kv, kw)
                    W2[:, gi, :] = w2[:, :, kt, ku, kv, kw].T
    return dict(X=Xs, MT=MTs, MB=MBs,
                W1P=W1P.reshape(128, 27 * 128), W1S=W1S.reshape(128, 27 * 128),
                W2=W2.reshape(128, 81 * 64),
                B1=np.asarray(b1, np.float32).reshape(128, 1),
                B2=np.asarray(b2, np.float32).reshape(64, 1))


def _build_module():
    import concourse.bass as bass
    import concourse.tile as tile
    from concourse import bacc, mybir

    fp16 = mybir.dt.float16
    fp32 = mybir.dt.float32

    nc = bacc.Bacc("TRN2", target_bir_lowering=False, debug=False, num_devices=1)
    x_d = nc.dram_tensor("x", [128, T, XP], fp16, kind="ExternalInput")
    w1p_d = nc.dram_tensor("w1p", [128, 27 * 128], fp16, kind="ExternalInput")
    w1s_d = nc.dram_tensor("w1s", [128, 27 * 128], fp16, kind="ExternalInput")
    w2_d = nc.dram_tensor("w2", [128, 81 * 64], fp16, kind="ExternalInput")
    b1_d = nc.dram_tensor("b1", [128, 1], fp32, kind="ExternalInput")
    b2_d = nc.dram_tensor("b2", [64, 1], fp32, kind="ExternalInput")
    mt_d = nc.dram_tensor("mt", [128, 1], fp32, kind="ExternalInput")
    mb_d = nc.dram_tensor("mb", [128, 1], fp32, kind="ExternalInput")
    y_d = nc.dram_tensor("y", [64, T, D * SH * W], fp32, kind="ExternalOutput")

    with tile.TileContext(nc) as tc:
        with (
            tc.tile_pool(name="xw", bufs=1) as xw,
            tc.tile_pool(name="hp", bufs=1) as hpool,
            tc.tile_pool(name="st", bufs=2) as stp,
            tc.tile_pool(name="p1", bufs=4, space="PSUM") as p1,
            tc.tile_pool(name="p2", bufs=4, space="PSUM") as p2,
        ):
            w1p = xw.tile([128, 27, 128], fp16)
            nc.sync.dma_start(w1p[:, :, :], w1p_d.ap())
            w1s = xw.tile([128, 27, 128], fp16)
            nc.sync.dma_start(w1s[:, :, :], w1s_d.ap())
            w2 = xw.tile([128, 81, 64], fp16)
            nc.sync.dma_start(w2[:, :, :], w2_d.ap())
            b1 = xw.tile([128, 1], fp32)
            nc.sync.dma_start(b1[:, :], b1_d.ap())
            b2 = xw.tile([64, 1], fp32)
            nc.sync.dma_start(b2[:, :], b2_d.ap())
            mt = xw.tile([128, 1], fp32)
            nc.sync.dma_start(mt[:, :], mt_d.ap())
            mb = xw.tile([128, 1], fp32)
            nc.sync.dma_start(mb[:, :], mb_d.ap())

            xt = xw.tile([128, T, XP], fp16)
            for t in range(T):
                nc.sync.dma_start(xt[:, t, :], x_d.ap()[:, t, :])

            ht = hpool.tile([128, T, HD, HHH, HW_], fp16)
            for t in range(T):
                nc.vector.memset(ht[:, t, :, :, :], 0.0)

            # ---- conv1 ----
            for t in range(T):
                groups = [(kt, ku, kv) for kt in _t_taps(t)
                          for ku in range(3) for kv in range(3)]
                for d in range(D):
                    ps = p1.tile([128, HHH, XROW], fp32)
                    n = len(groups)
                    # paired taps (kw=0 top + kw=1 bottom), K=128
                    for i, (kt, ku, kv) in enumerate(groups):
                        g = _g27(kt, ku, kv)
                        off = 1 + (d + ku) * XDP + kv * XROW - 1
                        nc.tensor.matmul(
                            ps[:, :, :], w1p[:, g, :],
                            xt[:, t + kt - 1, off:off + N1],
                            start=(i == 0), stop=False)
                    # single taps (kw=2), K=64
                    for i, (kt, ku, kv) in enumerate(groups):
                        g = _g27(kt, ku, kv)
                        off = 1 + (d + ku) * XDP + kv * XROW + 1
                        nc.tensor.matmul(
                            ps[:, :, :], w1s[0:64, g, :],
                            xt[0:64, t + kt - 1, off:off + N1],
                            start=False, stop=(i == n - 1))
                    nc.scalar.activation(
                        ht[:, t, d + 1, :, 1:33], ps[:, :, 1:33],
                        mybir.ActivationFunctionType.Relu, bias=b1[:, 0:1])
                # zero out-of-image h halo rows (mask is 0 only on edge cores)
                nc.vector.tensor_scalar_mul(
                    ht[:, t, :, 0, 1:33], ht[:, t, :, 0, 1:33], mt[:, 0:1])
                nc.vector.tensor_scalar_mul(
                    ht[:, t, :, HHH - 1, 1:33], ht[:, t, :, HHH - 1, 1:33],
                    mb[:, 0:1])

            # ---- conv2 ----
            for t in range(T):
                taps = [(kt, ku, kv, kw) for kt in _t_taps(t)
                        for ku in range(3) for kv in range(3) for kw in range(3)]
                st = stp.tile([64, D // 2, N2], fp32)
                n = len(taps)
                for dp in range(D // 2):
                    d0 = 2 * dp
                    ps = p2.tile([64, N2], fp32)
                    for i, (kt, ku, kv, kw) in enumerate(taps):
                        gi = _g81(kt, ku, kv, kw)
                        rhs = ht[:, t + kt - 1, d0 + ku:d0 + ku + 2,
                                 kv:kv + SH, kw:kw + W]
                        nc.tensor.matmul(ps[:, :], w2[:, gi, :], rhs,
                                         start=(i == 0), stop=(i == n - 1))
                    nc.vector.tensor_scalar_add(st[:, dp, :], ps[:, :], b2[:, 0:1])
                nc.sync.dma_start(y_d.ap()[:, t, :], st[:, :, :])
    nc.compile()
    return nc


def kernel(x, w1, b1, w2, b2):
    from concourse.bass_utils import run_bass_kernel_spmd

    hostd = _make_host_arrays(x, w1, b1, w2, b2)
    if "nc" not in _cache:
        _cache["nc"] = _build_module()
    nc = _cache["nc"]

    in_maps = []
    for core in range(NCORES):
        in_maps.append({
            "x": hostd["X"][core], "mt": hostd["MT"][core],
            "mb": hostd["MB"][core],
            "w1p": hostd["W1P"], "w1s": hostd["W1S"], "w2": hostd["W2"],
            "b1": hostd["B1"], "b2": hostd["B2"],
        })
    res = run_bass_kernel_spmd(nc, in_maps, core_ids=list(range(NCORES)))

    y = np.zeros((B, C_OUT, T, D, H, W), np.float32)
    for core in range(NCORES):
        b, j = divmod(core, NJ)
        yc = res.results[core]["y"].reshape(C_OUT, T, D, SH, W)
        y[b, :, :, :, SH * j:SH * (j + 1), :] = yc
    return y


# revision 7
# speedup vs baseline: 1.3696x; 1.3696x over previous
"""4D Conv-MLP (conv3^4 -> ReLU -> conv3^4) on 8 Trainium2 NeuronCores.

Sharding: core = b*4 + j  (batch b in {0,1}, H-slab j in {0..3}, 8 output rows
each). Each core computes its output slab independently: conv1 is recomputed on
a 1-row h halo (10 h rows from 12 x rows), so no cross-core communication is
needed. One SPMD program for all cores; per-core boundary behavior is driven by
data (host-zeroed x halos + h halo-row masks).

On-chip algorithm (implicit GEMM over the 81 taps, fp16 operands, fp32 PSUM):
  - x is stored channel-on-partition as a zero-padded flat plane per t:
    [18 D][12 H][34 W] (+1 lead pad), duplicated on partitions 64..127 shifted
    by one element so each K=128 matmul contracts two W-taps at once.
  - conv1: per (t, d): 340-column matmuls accumulating (kt,ku,kv) pair+single
    taps; ReLU+bias on the Scalar engine writes fp16 h (pads skipped).
  - conv2: per (t, d-pair): 512-column matmuls over all valid taps with
    K=128; bias added on DVE; fp32 result DMAd out.
"""

import numpy as np

B, C_IN, C_HID, C_OUT = 2, 64, 128, 64
T, D, H, W = 4, 16, 32, 32
NCORES, NJ = 8, 4
SH = H // NJ          # 8 out rows per slab
XH = SH + 4           # 12 x rows per slab
HHH = SH + 2          # 10 h rows per slab
XROW = 34             # padded W
XDP = 12 * XROW       # 408
XP = 1 + 18 * XDP + 7   # x plane size = 7352
HD, HW_ = 18, 34
HP = HD * HHH * HW_   # h plane = 6120
N1 = HHH * XROW       # conv1 run = 340
N2 = 512              # conv2 run (2 d-rows)

_cache = {}
import os
PAIR_CONV2 = os.environ.get('K_PAIR_CONV2', '1') == '1'
PAIR_SINGLES = os.environ.get('K_PAIR_SINGLES', '1') == '1'


def _t_taps(t):
    return [kt for kt in range(3) if 0 <= t + kt - 1 < T]


def _g27(kt, ku, kv):
    return (kt * 3 + ku) * 3 + kv


def _g81(kt, ku, kv, kw):
    return ((kt * 3 + ku) * 3 + kv) * 3 + kw


def _make_host_arrays(x, w1, b1, w2, b2):
    x = np.asarray(x, np.float32)
    Xs, MTs, MBs = [], [], []
    for core in range(NCORES):
        b, j = divmod(core, NJ)
        h0 = SH * j
        slab = np.zeros((C_IN, T, D, XH, W), np.float32)
        lo, hi = h0 - 2, h0 + 10
        slo, shi = max(lo, 0), min(hi, H)
        slab[:, :, :, slo - lo:shi - lo, :] = x[b, :, :, :, slo:shi, :]
        plane = np.zeros((C_IN, T, HD, XH, XROW), np.float32)
        plane[:, :, 1:17, :, 1:33] = slab
        flat = plane.reshape(C_IN, T, 18 * XDP)
        X = np.zeros((128, T, XP), np.float16)
        X[:64, :, 1:1 + 18 * XDP] = flat
        X[64:, :, 0:XP - 1] = X[:64, :, 1:XP]
        Xs.append(X)
        MTs.append(np.full((128, 1), 0.0 if j == 0 else 1.0, np.float32))
        MBs.append(np.full((128, 1), 0.0 if j == NJ - 1 else 1.0, np.float32))

    w1 = np.asarray(w1, np.float32)
    w2 = np.asarray(w2, np.float32)
    W1P = np.zeros((128, 27, 128), np.float16)
    W1S = np.zeros((128, 27, 128), np.float16)
    for kt in range(3):
        for ku in range(3):
            for kv in range(3):
                g = _g27(kt, ku, kv)
                W1P[:64, g, :] = w1[:, :, kt, ku, kv, 0].T
                W1P[64:, g, :] = w1[:, :, kt, ku, kv, 1].T
                W1S[:64, g, :] = w1[:, :, kt, ku, kv, 2].T
                W1S[64:, g, :] = w1[:, :, kt, ku, kv, 2].T
    W2 = np.zeros((128, 81, 64), np.float16)
    for kt in range(3):
        for ku in range(3):
            for kv in range(3):
                for kw in range(3):
                    gi = _g81(kt, ku, kv, kw)
                    W2[:, gi, :] = w2[:, :, kt, ku, kv, kw].T
    return dict(X=Xs, MT=MTs, MB=MBs,
                W1P=W1P.reshape(128, 27 * 128), W1S=W1S.reshape(128, 27 * 128),
                W2=W2.reshape(128, 81 * 64),
                B1=np.asarray(b1, np.float32).reshape(128, 1),
                B2=np.asarray(b2, np.float32).reshape(64, 1))


def _build_module():
    import concourse.bass as bass
    import concourse.tile as tile
    from concourse import bacc, mybir

    fp16 = mybir.dt.float16
    fp32 = mybir.dt.float32

    nc = bacc.Bacc("TRN2", target_bir_lowering=False, debug=False, num_devices=1)
    x_d = nc.dram_tensor("x", [128, T, XP], fp16, kind="ExternalInput")
    w1p_d = nc.dram_tensor("w1p", [128, 27 * 128], fp16, kind="ExternalInput")
    w1s_d = nc.dram_tensor("w1s", [128, 27 * 128], fp16, kind="ExternalInput")
    w2_d = nc.dram_tensor("w2", [128, 81 * 64], fp16, kind="ExternalInput")
    b1_d = nc.dram_tensor("b1", [128, 1], fp32, kind="ExternalInput")
    b2_d = nc.dram_tensor("b2", [64, 1], fp32, kind="ExternalInput")
    mt_d = nc.dram_tensor("mt", [128, 1], fp32, kind="ExternalInput")
    mb_d = nc.dram_tensor("mb", [128, 1], fp32, kind="ExternalInput")
    y_d = nc.dram_tensor("y", [64, T, D * SH * W], fp32, kind="ExternalOutput")

    with tile.TileContext(nc) as tc:
        with (
            tc.tile_pool(name="xw", bufs=1) as xw,
            tc.tile_pool(name="hp", bufs=1) as hpool,
            tc.tile_pool(name="st", bufs=2) as stp,
            tc.tile_pool(name="p1", bufs=4, space="PSUM") as p1,
            tc.tile_pool(name="p2", bufs=4, space="PSUM") as p2,
        ):
            w1p = xw.tile([128, 27, 128], fp16)
            nc.sync.dma_start(w1p[:, :, :], w1p_d.ap())
            w1s = xw.tile([128, 27, 128], fp16)
            nc.sync.dma_start(w1s[:, :, :], w1s_d.ap())
            w2 = xw.tile([128, 81, 64], fp16)
            nc.sync.dma_start(w2[:, :, :], w2_d.ap())
            b1 = xw.tile([128, 1], fp32)
            nc.sync.dma_start(b1[:, :], b1_d.ap())
            b2 = xw.tile([64, 1], fp32)
            nc.sync.dma_start(b2[:, :], b2_d.ap())
            mt = xw.tile([128, 1], fp32)
            nc.sync.dma_start(mt[:, :], mt_d.ap())
            mb = xw.tile([128, 1], fp32)
            nc.sync.dma_start(mb[:, :], mb_d.ap())

            xt = xw.tile([128, T, XP], fp16)
            for t in range(T):
                hxp = XP // 2
                nc.sync.dma_start(xt[:, t, 0:hxp], x_d.ap()[:, t, 0:hxp])
                nc.sync.dma_start(xt[:, t, hxp:XP], x_d.ap()[:, t, hxp:XP])

            ht = hpool.tile([128, T, HD, HHH, HW_], fp16)
            for t in range(T):
                nc.vector.memset(ht[:, t, :, :, :], 0.0)

            # ---- conv1 ----
            for t in range(T):
                for d in range(D):
                    # skip taps whose x D-row is an all-zero pad row
                    groups = [(kt, ku, kv) for kt in _t_taps(t)
                              for ku in range(3) for kv in range(3)
                              if 0 < d + ku < 17]
                    ps = p1.tile([128, HHH, XROW], fp32)
                    n = len(groups)
                    # paired taps (kw=0 top + kw=1 bottom), K=128
                    for i, (kt, ku, kv) in enumerate(groups):
                        g = _g27(kt, ku, kv)
                        off = 1 + (d + ku) * XDP + kv * XROW - 1
                        nc.tensor.matmul(
                            ps[:, :, :], w1p[:, g, :],
                            xt[:, t + kt - 1, off:off + N1],
                            start=(i == 0), stop=False)
                    # single taps (kw=2), K=64; alternate row groups
                    # (top copy / shifted bottom copy) so adjacent singles
                    # run concurrently in disjoint PE row groups
                    for i, (kt, ku, kv) in enumerate(groups):
                        g = _g27(kt, ku, kv)
                        off = 1 + (d + ku) * XDP + kv * XROW + 1
                        if i % 2 == 0 or not PAIR_SINGLES:
                            nc.tensor.matmul(
                                ps[:, :, :], w1s[0:64, g, :],
                                xt[0:64, t + kt - 1, off:off + N1],
                                start=False, stop=(i == n - 1))
                        else:
                            nc.tensor.matmul(
                                ps[:, :, :], w1s[64:128, g, :],
                                xt[64:128, t + kt - 1, off - 1:off - 1 + N1],
                                start=False, stop=(i == n - 1))
                    nc.scalar.activation(
                        ht[:, t, d + 1, :, 1:33], ps[:, :, 1:33],
                        mybir.ActivationFunctionType.Relu, bias=b1[:, 0:1])
                # zero out-of-image h halo rows (mask is 0 only on edge cores)
                nc.vector.tensor_scalar_mul(
                    ht[:, t, :, 0, 1:33], ht[:, t, :, 0, 1:33], mt[:, 0:1])
                nc.vector.tensor_scalar_mul(
                    ht[:, t, :, HHH - 1, 1:33], ht[:, t, :, HHH - 1, 1:33],
                    mb[:, 0:1])

            # ---- conv2 ----
            # taps alternate between PE column groups (psum partitions 0:64 /
            # 64:128) so adjacent matmuls run concurrently; halves summed on DVE
            for t in range(T):
                taps = [(kt, ku, kv, kw) for kt in _t_taps(t)
                        for ku in range(3) for kv in range(3) for kw in range(3)]
                st = stp.tile([64, D // 2, N2], fp32)
                lo = taps[0::2]
                hi = taps[1::2]
                for dp in range(D // 2):
                    d0 = 2 * dp
                    if PAIR_CONV2:
                        ps = p2.tile([128, N2], fp32)
                        for i in range(len(lo)):
                            for half, base, tp_pos in ((lo, 0, (0, 0)),
                                                       (hi, 64, (0, 64))):
                                if i >= len(half):
                                    continue
                                kt, ku, kv, kw = half[i]
                                gi = _g81(kt, ku, kv, kw)
                                rhs = ht[:, t + kt - 1, d0 + ku:d0 + ku + 2,
                                         kv:kv + SH, kw:kw + W]
                                nc.tensor.matmul(
                                    ps[base:base + 64, :], w2[:, gi, :], rhs,
                                    start=(i == 0), stop=(i == len(half) - 1),
                                    tile_position=tp_pos)
                        nc.scalar.activation(
                            st[:, dp, :], ps[64:128, :],
                            mybir.ActivationFunctionType.Identity, bias=b2[:, 0:1])
                        nc.vector.tensor_add(st[:, dp, :], st[:, dp, :],
                                             ps[0:64, :])
                    else:
                        ps = p2.tile([64, N2], fp32)
                        n = len(taps)
                        for i, (kt, ku, kv, kw) in enumerate(taps):
                            gi = _g81(kt, ku, kv, kw)
                            rhs = ht[:, t + kt - 1, d0 + ku:d0 + ku + 2,
                                     kv:kv + SH, kw:kw + W]
                            nc.tensor.matmul(ps[:, :], w2[:, gi, :], rhs,
                                             start=(i == 0), stop=(i == n - 1))
                        nc.vector.tensor_scalar_add(st[:, dp, :], ps[:, :],
                                                    b2[:, 0:1])
                nc.sync.dma_start(y_d.ap()[:, t, :], st[:, :, :])
    nc.compile()
    return nc


def kernel(x, w1, b1, w2, b2):
    from concourse.bass_utils import run_bass_kernel_spmd

    hostd = _make_host_arrays(x, w1, b1, w2, b2)
    if "nc" not in _cache:
        _cache["nc"] = _build_module()
    nc = _cache["nc"]

    in_maps = []
    for core in range(NCORES):
        in_maps.append({
            "x": hostd["X"][core], "mt": hostd["MT"][core],
            "mb": hostd["MB"][core],
            "w1p": hostd["W1P"], "w1s": hostd["W1S"], "w2": hostd["W2"],
            "b1": hostd["B1"], "b2": hostd["B2"],
        })
    res = run_bass_kernel_spmd(nc, in_maps, core_ids=list(range(NCORES)))

    y = np.zeros((B, C_OUT, T, D, H, W), np.float32)
    for core in range(NCORES):
        b, j = divmod(core, NJ)
        yc = res.results[core]["y"].reshape(C_OUT, T, D, SH, W)
        y[b, :, :, :, SH * j:SH * (j + 1), :] = yc
    return y
